# revision 5
# baseline (speedup 1.0000x reference)
"""Multi-head self-attention Trainium2 Bass kernel.

Problem (hardcoded): x (2, 2048, 512) fp32, 8 heads of dim 64,
torch-Linear q/k/v/o projections (y = x @ W.T + b).

Sharding: 8 cores = 2 batches x 4 query-chunks of 512. Each core
computes K/V for its whole batch (replicated across the 4 cores of the
batch) and attention + output projection for its own 512 queries.

Host-side prep (free for the device): per-batch x is passed transposed
(xT [512, 2048]) and weights pre-transposed (wT = W.T), cast to bf16.
The kernel writes yT [512 o, 512 q] fp32; the host transposes back.

Design (HW-microbenchmark driven; measured per-instruction costs):
 - ACT exp is the hard floor: 64 activations of [128, 1024] at ~752 ns
   = ~48 us/iter (dtype-independent 1.2 GHz spline engine; FD=1024 is
   the measured sweet spot). Everything else is scheduled to keep ACT
   back-to-back.
 - Scores: head pair (2c, 2c+1) on PE row groups 0-63/64-127 runs
   concurrently (measured 114 ns/pair at N=512 vs 132 ns for one
   full-K matmul).
 - AV: fp8e4m3 DoubleRow, contraction 256 (two key tiles per matmul,
   measured 123 ns) - halves AV instruction count vs bf16. e (softmax
   numerator) is written by ACT directly in fp8; V is stored fp8 with
   a ones column so PSUM row 64 accumulates the denominator for free.
   fp8 noise (~3%/element) averages out over n_eff ~ 1.8k keys
   (scores sigma ~ 0.33) -> ~0.1% output error. Scores/projections
   stay bf16 (fp8 there would put ~5% noise on pre-softmax logits).
 - All biases are folded into the projection matmuls as rank-1
   accumulating matmuls (bias row x ones row), so every PSUM
   evacuation is a pure DVE copy (measured 307 ns for [128,512]
   f32->bf16 vs 439 ns for the tensor_scalar_add it replaces).
 - Emission scheduling: the PE queue is in-order, so the scores
   matmuls (which gate exp) must never sit behind bulk work. Each
   slot emits: scores pair -> exp -> (one fp8 AV pair, held back >= 2
   slots, so its exp-wait is long satisfied) -> paced filler
   (projection-chain matmuls at single-matmul granularity, ~420
   ns/slot budget, with deadline forcing before first consumers).
   Chain work for the next iteration starts draining at slot 51 so
   iteration boundaries add no ACT bubble.
 - Softmax normalize: denominator copies first (DVE), rank-1
   denominator-broadcast matmuls on disjoint PE row groups, DVE
   reciprocal, multiplies on the otherwise-idle Pool engine.
 - PSUM budget 8 banks: proj 2 + scores 2x2 + av 2.
"""

import numpy as np

import concourse.bass as bass
import concourse.mybir as mybir
import concourse.tile as tile
from concourse.bass_utils import run_bass_kernel_spmd

B = 2
S = 2048
D = 512
H = 8
DH = 64
QC = 512  # queries per core
N_CORES = 8
P = 128
DC = D // P  # 4 contraction / output chunks
KT_TILES = S // P  # 16 key tiles
KTP = KT_TILES // 2  # 8 key-tile pairs (fp8 DoubleRow contraction 256)
HC = H // 2  # 4 head pairs
F32 = mybir.dt.float32
BF16 = mybir.dt.bfloat16
FP8 = mybir.dt.float8e4
VW = DH + 2  # V row width: 64 values + ones col + pad (fp8 DR needs
#              the Ko stride 8*VW to be a multiple of 16 bytes)

# measured per-instruction PE costs (ns) used only for pacing
MM_NS = 132.0
BIAS_NS = 115.0


def _split_waits(nc: bass.Bass, max_waits: int = 1):
    """walrus encodes at most one sync-wait on several S3 instruction
    structs (fused-load Matmult, TensorScalarPtr, Activation, ...). Hoist
    excess waits onto same-engine NoOps inserted immediately before the
    instruction — sequencer order preserves semantics."""
    eng_map = {
        mybir.EngineType.PE: lambda: nc.tensor,
        mybir.EngineType.DVE: lambda: nc.vector,
        mybir.EngineType.Activation: lambda: nc.scalar,
        mybir.EngineType.Pool: lambda: nc.gpsimd,
        mybir.EngineType.SP: lambda: nc.sync,
    }
    for f in nc.m.functions:
        for blk in f.blocks:
            insts = list(blk.instructions)
            out = []
            changed = False
            for inst in insts:
                si = inst.sync_info
                if (
                    si is not None
                    and si.on_wait
                    and len(si.on_wait) > max_waits
                    and inst.engine in eng_map
                ):
                    waits = list(si.on_wait)
                    keep = waits[:max_waits]
                    extra = waits[max_waits:]
                    eng = eng_map[inst.engine]()
                    for w in extra:
                        nop = eng.nop().ins
                        cur = nc.cur_bb.bb
                        cur_insts = list(cur.instructions)
                        assert cur_insts and cur_insts[-1].name == nop.name
                        cur.instructions = cur_insts[:-1]
                        nop.sync_info = mybir.SyncInfo(on_wait=[w], on_update=[])
                        out.append(nop)
                    inst.sync_info = mybir.SyncInfo(
                        on_wait=keep, on_update=list(si.on_update or [])
                    )
                    changed = True
                out.append(inst)
            if changed:
                blk.instructions = out


def build_nc(iters: int = 1) -> bass.Bass:
    """Build the single-core SPMD Bass program (same program, all cores)."""
    nc = bass.Bass()

    xT = nc.dram_tensor("xT", [D, S], BF16, kind="ExternalInput")
    xTq = nc.dram_tensor("xTq", [D, QC], BF16, kind="ExternalInput")
    wqT = nc.dram_tensor("wqT", [D, D], BF16, kind="ExternalInput")
    wkT = nc.dram_tensor("wkT", [D, D], BF16, kind="ExternalInput")
    wvT = nc.dram_tensor("wvT", [D, D], BF16, kind="ExternalInput")
    woT = nc.dram_tensor("woT", [D, D], BF16, kind="ExternalInput")
    bq = nc.dram_tensor("bq", [D], BF16, kind="ExternalInput")
    bk = nc.dram_tensor("bk", [D], BF16, kind="ExternalInput")
    bv = nc.dram_tensor("bv", [D], BF16, kind="ExternalInput")
    bo = nc.dram_tensor("bo", [D], BF16, kind="ExternalInput")
    ones128 = nc.dram_tensor("ones128", [P], BF16, kind="ExternalInput")
    ones512 = nc.dram_tensor("ones512", [QC], BF16, kind="ExternalInput")
    vinit = nc.dram_tensor("vinit", [2], FP8, kind="ExternalInput")
    yT = nc.dram_tensor("yT", [D, QC], F32, kind="ExternalOutput")

    with tile.TileContext(nc) as tc:
        with (
            nc.allow_low_precision(reason="bf16/fp8 matmul operands"),
            tc.tile_pool(name="const", bufs=1) as const_pool,
            tc.tile_pool(name="acts", bufs=1) as acts_pool,
            tc.tile_pool(name="e", bufs=3) as e_pool,
            tc.tile_pool(name="small", bufs=8) as small_pool,
            tc.tile_pool(name="avsb", bufs=6) as avsb_pool,
            tc.tile_pool(name="work_ps", bufs=2, space="PSUM") as proj_ps,
            tc.tile_pool(name="score_ps", bufs=2, space="PSUM") as score_ps,
            tc.tile_pool(name="av_ps", bufs=2, space="PSUM") as av_ps,
        ):
            # ---- tiny constants first ----
            brow = {}
            for name, t in (("q", bq), ("k", bk), ("v", bv), ("o", bo)):
                r = const_pool.tile([1, D], BF16, tag=f"b{name}")
                nc.sync.dma_start(out=r, in_=t.rearrange("(o d) -> o d", o=1))
                brow[name] = r
            ones_sb = const_pool.tile([1, QC], BF16, tag="ones512")
            nc.sync.dma_start(
                out=ones_sb, in_=ones512.rearrange("(o d) -> o d", o=1)
            )
            ones65 = const_pool.tile([DH + 1, P], BF16, tag="ones65")
            nc.sync.dma_start(
                out=ones65,
                in_=ones128.rearrange("(o d) -> o d", o=1).broadcast_to(
                    [DH + 1, P]
                ),
            )

            # ---- bulk inputs, ordered so Q's operands land first ----
            xTq_sb = acts_pool.tile([P, DC, QC], BF16, tag="xTq")
            nc.sync.dma_start(out=xTq_sb, in_=xTq.rearrange("(c p) t -> p c t", p=P))
            w_sb = {}
            for name, t in (("q", wqT), ("k", wkT)):
                w = const_pool.tile([P, DC, D], BF16, tag=f"w{name}")
                nc.sync.dma_start(out=w, in_=t.rearrange("(c p) o -> p c o", p=P))
                w_sb[name] = w
            xT_sb = acts_pool.tile([P, DC, S], BF16, tag="xT")
            xT_r = xT.rearrange("(c p) t -> p c t", p=P)
            for tc_ in range(DC):
                nc.sync.dma_start(
                    out=xT_sb[:, :, tc_ * QC : (tc_ + 1) * QC],
                    in_=xT_r[:, :, tc_ * QC : (tc_ + 1) * QC],
                )
            for name, t in (("v", wvT), ("o", woT)):
                w = const_pool.tile([P, DC, D], BF16, tag=f"w{name}")
                nc.sync.dma_start(out=w, in_=t.rearrange("(c p) o -> p c o", p=P))
                w_sb[name] = w

            # ---- persistent activation tiles (hoisted out of the loop) ----
            QT_sb = acts_pool.tile([P, DC, QC], BF16, tag="QT")
            KT_sb = acts_pool.tile([P, DC, S], BF16, tag="KT")
            V8_sb = acts_pool.tile([P, KTP, 2, H, VW], FP8, tag="V8")
            OUT_sb = acts_pool.tile([P, DC, QC], BF16, tag="OUT")
            yT_sb = acts_pool.tile([P, DC, QC], F32, tag="yT")
            # ones column (softmax denominator) + zero pad — written ONCE
            # (V-proj only ever rewrites cols 0..63)
            nc.sync.dma_start(
                out=V8_sb.rearrange("p a b c d -> p (a b c) d")[
                    :, :, DH : DH + 2
                ],
                in_=vinit.rearrange("(o d) -> o d", o=1)
                .unsqueeze(0)
                .broadcast_to([P, KTP * 2 * H, 2]),
            )

            # ---- projection chains as micro-item lists ----
            def q_chain_items(ot):
                ps_ref = []

                def mk_mm(dc):
                    def f():
                        if dc == 0:
                            ps_ref.append(
                                proj_ps.tile([P, QC], F32, tag="proj", name="psq")
                            )
                        nc.tensor.matmul(
                            ps_ref[0],
                            w_sb["q"][:, dc, ot * P : (ot + 1) * P],
                            xTq_sb[:, dc, :],
                            start=(dc == 0),
                            stop=False,
                        )

                    return f

                def bias():
                    nc.tensor.matmul(
                        ps_ref[0],
                        brow["q"][:, ot * P : (ot + 1) * P],
                        ones_sb,
                        start=False,
                        stop=True,
                    )

                def evac():
                    nc.vector.tensor_copy(out=QT_sb[:, ot, :], in_=ps_ref[0])

                return [(mk_mm(dc), MM_NS) for dc in range(DC)] + [
                    (bias, BIAS_NS),
                    (evac, 0.0),
                ]

            def k_chain_items(tc_, ot):
                ps_ref = []

                def mk_mm(dc):
                    def f():
                        if dc == 0:
                            ps_ref.append(
                                proj_ps.tile([P, QC], F32, tag="proj", name="psk")
                            )
                        nc.tensor.matmul(
                            ps_ref[0],
                            w_sb["k"][:, dc, ot * P : (ot + 1) * P],
                            xT_sb[:, dc, tc_ * QC : (tc_ + 1) * QC],
                            start=(dc == 0),
                            stop=False,
                        )

                    return f

                def bias():
                    nc.tensor.matmul(
                        ps_ref[0],
                        brow["k"][:, ot * P : (ot + 1) * P],
                        ones_sb,
                        start=False,
                        stop=True,
                    )

                def evac():
                    nc.vector.tensor_copy(
                        out=KT_sb[:, ot, tc_ * QC : (tc_ + 1) * QC], in_=ps_ref[0]
                    )

                return [(mk_mm(dc), MM_NS) for dc in range(DC)] + [
                    (bias, BIAS_NS),
                    (evac, 0.0),
                ]

            def v_chain_items(tt):
                ps_ref = []

                def mk_mm(dc):
                    def f():
                        if dc == 0:
                            ps_ref.append(
                                proj_ps.tile([P, D], F32, tag="proj", name="psv")
                            )
                        nc.tensor.matmul(
                            ps_ref[0],
                            xT_sb[:, dc, tt * P : (tt + 1) * P],
                            w_sb["v"][:, dc, :],
                            start=(dc == 0),
                            stop=False,
                        )

                    return f

                def bias():
                    nc.tensor.matmul(
                        ps_ref[0],
                        ones_sb[:, 0:P],
                        brow["v"],
                        start=False,
                        stop=True,
                    )

                def evac():
                    nc.vector.tensor_copy(
                        out=V8_sb[:, tt // 2, tt % 2, :, 0:DH],
                        in_=ps_ref[0].rearrange("p (h j) -> p h j", h=H),
                    )

                return [(mk_mm(dc), MM_NS) for dc in range(DC)] + [
                    (bias, BIAS_NS),
                    (evac, 0.0),
                ]

            def o_chain_items(ot):
                ps_ref = []

                def mk_mm(dc):
                    def f():
                        if dc == 0:
                            ps_ref.append(
                                proj_ps.tile([P, QC], F32, tag="proj", name="pso")
                            )
                        nc.tensor.matmul(
                            ps_ref[0],
                            w_sb["o"][:, dc, ot * P : (ot + 1) * P],
                            OUT_sb[:, dc, :],
                            start=(dc == 0),
                            stop=False,
                        )

                    return f

                def bias():
                    nc.tensor.matmul(
                        ps_ref[0],
                        brow["o"][:, ot * P : (ot + 1) * P],
                        ones_sb,
                        start=False,
                        stop=True,
                    )

                def evac():
                    nc.vector.tensor_copy(out=yT_sb[:, ot, :], in_=ps_ref[0])

                return [(mk_mm(dc), MM_NS) for dc in range(DC)] + [
                    (bias, BIAS_NS),
                    (evac, 0.0),
                ]

            # ---- attention slot pieces ----
            def scores_exp(hc, kt, e_t):
                s_ps = score_ps.tile([P, 2, QC], F32, tag="score")
                for j in range(2):
                    hp = j * DH
                    nc.tensor.matmul(
                        s_ps[:, j, :],
                        KT_sb[hp : hp + DH, hc, kt * P : (kt + 1) * P],
                        QT_sb[hp : hp + DH, hc, :],
                        start=True,
                        stop=True,
                    )
                nc.scalar.activation(
                    out=e_t[:, kt % 2, :, :],
                    in_=s_ps,
                    func=mybir.ActivationFunctionType.Exp,
                    scale=0.125,
                )

            def av_emit(hc, ktp, av_pair, e_t):
                for j in range(2):
                    nc.tensor.matmul(
                        av_pair[j],
                        V8_sb[:, ktp, :, 2 * hc + j, :],
                        e_t[:, :, j, :],
                        start=(ktp == 0),
                        stop=(ktp == KTP - 1),
                        perf_mode=mybir.MatmulPerfMode.DoubleRow,
                    )

            def fin_evac(hc, av_pair):
                """Tiny den copies FIRST (so the bc matmul unblocks fast),
                then evacuate the AV PSUM pair to SBUF."""
                dens, accs = [], []
                den0 = small_pool.tile([1, QC], BF16, tag="den")
                nc.vector.tensor_copy(out=den0, in_=av_pair[0][DH : DH + 1, :])
                dens.append(den0)
                den1 = small_pool.tile([DH + 1, QC], BF16, tag="den1")
                nc.vector.tensor_copy(
                    out=den1[DH : DH + 1, :], in_=av_pair[1][DH : DH + 1, :]
                )
                dens.append(den1[DH : DH + 1, :])
                for j in range(2):
                    acc = avsb_pool.tile([DH, QC], BF16, tag="avsb")
                    nc.vector.tensor_copy(out=acc, in_=av_pair[j][0:DH, :])
                    accs.append(acc)
                return dens, accs

            def fin_norm(hc, dens, accs):
                """Denominator-broadcast matmuls on PE row groups 0/64,
                reciprocal on DVE, multiplies on the Pool engine."""
                bcs = []
                for j in range(2):
                    bc = proj_ps.tile([DH, QC], F32, tag="proj")
                    lhs = (
                        ones65[0:1, 0:DH]
                        if j == 0
                        else ones65[DH : DH + 1, 0:DH]
                    )
                    nc.tensor.matmul(bc, lhs, dens[j], start=True, stop=True)
                    bcs.append(bc)
                for j in range(2):
                    hp = j * DH
                    rec64 = small_pool.tile([DH, QC], F32, tag="rec64")
                    nc.vector.reciprocal(out=rec64, in_=bcs[j])
                    nc.gpsimd.tensor_mul(
                        out=OUT_sb[hp : hp + DH, hc, :],
                        in0=accs[j],
                        in1=rec64,
                    )

            # ---- scheduler state spanning iterations ----
            fifo = []  # (fn, pe_cost)
            drained = [0]  # index of next undrained item
            spent = [0.0]  # pe-cost drained so far
            target = [0.0]

            def drain_to(idx):
                while drained[0] < min(idx, len(fifo)):
                    fn, c = fifo[drained[0]]
                    fn()
                    spent[0] += c
                    drained[0] += 1

            def drain_paced():
                while drained[0] < len(fifo) and spent[0] < target[0]:
                    fn, c = fifo[drained[0]]
                    fn()
                    spent[0] += c
                    drained[0] += 1

            def drain_all():
                drain_to(len(fifo))

            PACE_NS = 430.0

            # marks for the CURRENT iteration: g -> fifo index
            marks = {}

            def append_items(items, due_g=None, m=None):
                fifo.extend(items)
                if due_g is not None:
                    (m if m is not None else marks)[due_g] = len(fifo)

            def iteration_body(first: bool, last: bool):
                nonlocal marks
                cur_marks = marks
                marks = {}
                if first:
                    # no previous iteration primed the boundary chains
                    append_items(q_chain_items(0), 0, cur_marks)
                    for ot in range(DC):
                        append_items(k_chain_items(0, ot), 0, cur_marks)
                    append_items(
                        v_chain_items(0) + v_chain_items(1), 3, cur_marks
                    )
                # rest of this iteration's chains, in deadline order
                for ot in range(DC):
                    append_items(k_chain_items(1, ot), 4, cur_marks)
                append_items(v_chain_items(2) + v_chain_items(3), 5, cur_marks)
                append_items(v_chain_items(4) + v_chain_items(5), 7, cur_marks)
                for ot in range(DC):
                    append_items(k_chain_items(2, ot), 8, cur_marks)
                append_items(v_chain_items(6) + v_chain_items(7), 9, cur_marks)
                append_items(v_chain_items(8) + v_chain_items(9), 11, cur_marks)
                for ot in range(DC):
                    append_items(k_chain_items(3, ot), 12, cur_marks)
                append_items(
                    v_chain_items(10) + v_chain_items(11), 13, cur_marks
                )
                append_items(
                    v_chain_items(12) + v_chain_items(13), 15, cur_marks
                )
                append_items(q_chain_items(1), 16, cur_marks)
                append_items(
                    v_chain_items(14) + v_chain_items(15), 17, cur_marks
                )
                append_items(q_chain_items(2), 32, cur_marks)
                append_items(q_chain_items(3), 48, cur_marks)

                pending = []  # (hc, ktp, av_pair, e_t, pushed_g)
                deferred = []  # (hc, dens, accs) for fin_norm
                av_pair = None
                e_t = None
                for g in range(HC * KT_TILES):
                    hc, kt = divmod(g, KT_TILES)
                    if g in cur_marks:
                        drain_to(cur_marks[g])
                    if kt == 0:
                        av_pair = [
                            av_ps.tile(
                                [DH + 2, QC], F32, tag="av", name=f"avp{g}_{j}"
                            )
                            for j in range(2)
                        ]
                    if kt % 2 == 0:
                        e_t = e_pool.tile([P, 2, 2, QC], FP8, tag="e")
                    scores_exp(hc, kt, e_t)
                    if kt % 2 == 1:
                        pending.append((hc, (kt - 1) // 2, av_pair, e_t, g))
                    while pending and pending[0][4] <= g - 2:
                        h2, ktp2, ap2, et2, _ = pending.pop(0)
                        av_emit(h2, ktp2, ap2, et2)
                        if ktp2 == KTP - 1:
                            dens, accs = fin_evac(h2, ap2)
                            deferred.append((h2, dens, accs))
                    if deferred and kt >= 3 and kt % 2 == 1:
                        f = deferred.pop(0)
                        append_items(
                            [(lambda f=f: fin_norm(*f), 2 * BIAS_NS)]
                        )
                    if g == 51 and not last:
                        # prime next iteration's boundary chains
                        append_items(q_chain_items(0), 64 + 0, marks)
                        for ot in range(DC):
                            append_items(k_chain_items(0, ot), 64 + 0, marks)
                        append_items(
                            v_chain_items(0) + v_chain_items(1), 64 + 3, marks
                        )
                    target[0] += PACE_NS
                    drain_paced()
                # iteration tail: flush leftover AV (hc=3, ktp=7), its
                # finalize, O chains and the output DMA into the fifo;
                # they drain in the next iteration's early slots.
                while pending:
                    h2, ktp2, ap2, et2, _ = pending.pop(0)

                    def flush_av(h2=h2, ktp2=ktp2, ap2=ap2, et2=et2):
                        av_emit(h2, ktp2, ap2, et2)

                    append_items([(flush_av, 2 * MM_NS)])
                    if ktp2 == KTP - 1:

                        def fin_item(h2=h2, ap2=ap2):
                            dens, accs = fin_evac(h2, ap2)
                            fin_norm(h2, dens, accs)

                        append_items([(fin_item, 2 * BIAS_NS)], 1, marks)
                for f in deferred:
                    append_items([(lambda f=f: fin_norm(*f), 2 * BIAS_NS)])
                for ot in range(DC):
                    append_items(o_chain_items(ot))
                append_items(
                    [
                        (
                            lambda: nc.sync.dma_start(
                                out=yT.rearrange("(c p) q -> p c q", p=P),
                                in_=yT_sb,
                            ),
                            0.0,
                        )
                    ]
                )
                # re-key next-iteration marks from 64+g to g
                marks = {k - 64 if k >= 64 else k: v for k, v in marks.items()}

            for i in range(iters):
                iteration_body(first=(i == 0), last=(i == iters - 1))
            drain_all()

    _split_waits(nc)
    return nc


def make_in_maps(x, wq, bq, wk, bk, wv, bv, wo, bo):
    """Host-side sharding: per-core input dicts (bf16/fp8 operands)."""
    import ml_dtypes

    BF = ml_dtypes.bfloat16
    F8 = ml_dtypes.float8_e4m3
    x = np.asarray(x, dtype=np.float32)
    xT_b = [np.ascontiguousarray(x[b].T).astype(BF) for b in range(B)]
    wT = {
        "wqT": np.ascontiguousarray(np.asarray(wq, np.float32).T).astype(BF),
        "wkT": np.ascontiguousarray(np.asarray(wk, np.float32).T).astype(BF),
        "wvT": np.ascontiguousarray(np.asarray(wv, np.float32).T).astype(BF),
        "woT": np.ascontiguousarray(np.asarray(wo, np.float32).T).astype(BF),
    }
    biases = {
        "bq": np.asarray(bq, np.float32).astype(BF),
        "bk": np.asarray(bk, np.float32).astype(BF),
        "bv": np.asarray(bv, np.float32).astype(BF),
        "bo": np.asarray(bo, np.float32).astype(BF),
        "ones128": np.ones(P, BF),
        "ones512": np.ones(QC, BF),
        "vinit": np.array([1.0, 0.0], F8),
    }
    in_maps = []
    for c in range(N_CORES):
        b, qc = divmod(c, N_CORES // B)
        in_maps.append(
            {
                "xT": xT_b[b],
                "xTq": np.ascontiguousarray(xT_b[b][:, qc * QC : (qc + 1) * QC]),
                **wT,
                **biases,
            }
        )
    return in_maps


def assemble_output(results):
    y = np.empty((B, S, D), dtype=np.float32)
    for c in range(N_CORES):
        b, qc = divmod(c, N_CORES // B)
        y[b, qc * QC : (qc + 1) * QC, :] = results[c]["yT"].T
    return y


def kernel(**inputs) -> np.ndarray:
    nc = build_nc()
    in_maps = make_in_maps(**inputs)
    res = run_bass_kernel_spmd(nc, in_maps, list(range(N_CORES)))
    return assemble_output(res.results)


if __name__ == "__main__":
    rng = np.random.default_rng(0)
    s = 1.0 / np.sqrt(D)
    inputs = {
        "x": rng.standard_normal((B, S, D), dtype=np.float32),
        "wq": rng.uniform(-s, s, (D, D)).astype(np.float32),
        "bq": rng.uniform(-s, s, D).astype(np.float32),
        "wk": rng.uniform(-s, s, (D, D)).astype(np.float32),
        "bk": rng.uniform(-s, s, D).astype(np.float32),
        "wv": rng.uniform(-s, s, (D, D)).astype(np.float32),
        "bv": rng.uniform(-s, s, D).astype(np.float32),
        "wo": rng.uniform(-s, s, (D, D)).astype(np.float32),
        "bo": rng.uniform(-s, s, D).astype(np.float32),
    }
    y = kernel(**inputs)
    print("output", y.shape, y.dtype)
